# revision 1
# baseline (speedup 1.0000x reference)
"""GNN message-passing edge scorer on 8 TRN2 NeuronCores.

Model: out[e] = relu(concat(U[src[e]], M[dst[e]]) @ W1 + b1) @ W2 + b2
  U, M: [100000, 128] f32 node tables; edge_index: [2, 1000000] int32/64.

v2 strategy (edge-parallel, tables replicated; algebraic restructure):
  relu(u@W1top + m@W1bot + b1) @ W2 + b2
  - NEFF-A (8-core SPMD): each core computes its 1/8 shard of the
    PRE-MIXED tables A = U @ W1top and B' = M @ W1bot + b1 (dense PE
    matmuls, f32->fp16 cast-DMA load, PE transpose for the contraction
    layout), written node-major fp16.  Host concatenates shards.
  - NEFF-B (8-core SPMD): per 512-edge block, dma_gather(transpose=False,
    4 SWDGE queues round-robin) pulls A[src]/B'[dst] rows EDGE-major into
    SBUF; DVE adds them, ACT applies relu, DVE tensor_tensor_reduce
    fuses (h * W2bcast, +reduce, +b2) -> one f32 per edge.
  - Host buckets each core's 125k edges into 16 buckets by
    (src//25088, dst//25088) so chunk-local indices fit int16, and
    applies the inverse permutation to assemble the output.

HW facts this design is built on (measured on this runtime):
  - dma_gather descriptor path is the bottleneck: 9.4 ns/row on one
    queue regardless of source (HBM or SBUF) or elem size; 4 queues
    (num_swdge_queues=4, transpose=False) -> 3.5 ns/row, data-valid.
  - transpose=True multi-queue gathers return corrupted data; the
    single-queue transpose=True path also forces xbar fencing vs plain
    DMA.  transpose=False avoids both.
  - Plain/contiguous DMA runs ~190-360 GB/s: dense table precompute is
    nearly free compared to per-edge gathers.
"""

import numpy as np

N_NODES = 100000
H = 128
N_CORES = 8
SHARD = 12544                 # NEFF-A rows per core (98 * 128)
NB_A = SHARD // 128           # 98 blocks per table per core
NP = SHARD * N_CORES          # 100352 padded table rows
N_CHUNKS = 4
CHUNK = NP // N_CHUNKS        # 25088, int16-addressable
N_BUCKETS = N_CHUNKS * N_CHUNKS
BLK = 512                     # edges per compute block
MAX_CALL = 4096               # indices per dma_gather call
N_QUEUES = 4                  # SWDGE queues for gathers

_cache = {}


def _build_neff_a(reps=1):
    import concourse.bacc as bacc
    import concourse.mybir as mybir
    import concourse.tile as tile

    f32 = mybir.dt.float32
    fp16 = mybir.dt.float16
    ACT = mybir.ActivationFunctionType
    ALU = mybir.AluOpType

    nc = bacc.Bacc("TRN2", target_bir_lowering=False, debug=False,
                   num_devices=N_CORES)
    ush = nc.dram_tensor("ush", [SHARD, H], f32, kind="ExternalInput")
    msh = nc.dram_tensor("msh", [SHARD, H], f32, kind="ExternalInput")
    wk = nc.dram_tensor("wk", [128, 3 * H], fp16, kind="ExternalInput")
    b1bc = nc.dram_tensor("b1bc", [128, H], f32, kind="ExternalInput")
    a16o = nc.dram_tensor("a16o", [SHARD, H], fp16, kind="ExternalOutput")
    b16o = nc.dram_tensor("b16o", [SHARD, H], fp16, kind="ExternalOutput")

    with tile.TileContext(nc) as tc:
        with (
            tc.tile_pool(name="src", bufs=2) as spool,
            tc.tile_pool(name="stg", bufs=2) as stpool,
            tc.tile_pool(name="w", bufs=1) as wpool,
            tc.tile_pool(name="ut", bufs=4) as utpool,
            tc.tile_pool(name="pt", bufs=3, space="PSUM") as ptpool,
            tc.tile_pool(name="pa", bufs=3, space="PSUM") as papool,
        ):
            wsb = wpool.tile([128, 3 * H], fp16, tag="wsb")
            nc.sync.dma_start(wsb[:], wk[:])
            bsb = wpool.tile([128, H], f32, tag="bsb")
            nc.sync.dma_start(bsb[:], b1bc[:])
            ident = wsb[:, 2 * H:3 * H]

            def one_table(src_d, out_d, wcol, is_b):
                # cast-load: usb[p, nb*H+f] = src[p*NB_A + nb, f] in fp16
                usb = spool.tile([128, NB_A * H], fp16, tag="usb")
                nc.gpsimd.dma_start(usb[:], src_d[:])
                stg = stpool.tile([128, NB_A * H], fp16, tag="stg")
                for nb in range(NB_A):
                    blk_in = usb[:, nb * H:(nb + 1) * H]
                    pt = ptpool.tile([128, H], fp16, tag="pt")
                    nc.tensor.transpose(pt[:], blk_in, ident)
                    ut = utpool.tile([128, H], fp16, tag="ut")
                    nc.scalar.activation(ut[:], pt[:], ACT.Copy)
                    pa = papool.tile([128, H], f32, tag="pa")
                    nc.tensor.matmul(pa[:], ut[:], wsb[:, wcol * H:(wcol + 1) * H],
                                     start=True, stop=True)
                    dst = stg[:, nb * H:(nb + 1) * H]
                    if is_b:
                        nc.vector.scalar_tensor_tensor(
                            dst, pa[:], 0.0, bsb[:], op0=ALU.add, op1=ALU.add)
                    else:
                        nc.scalar.activation(dst, pa[:], ACT.Copy)
                nc.sync.dma_start(out_d[:], stg[:])

            def body():
                one_table(ush, a16o, 0, False)
                one_table(msh, b16o, 1, True)

            body()
            if reps > 1:
                with tc.For_i(0, reps - 1):
                    body()
    nc.compile()
    return nc


def _build_neff_b(b_sizes, reps=1):
    """b_sizes: list of 16 padded bucket sizes (multiples of BLK, may be 0).

    Output layout: out [128, out_cols] f32; padded-stream slot s lands at
    out[s % 128, s // 128].
    """
    import concourse.bacc as bacc
    import concourse.mybir as mybir
    import concourse.tile as tile

    f32 = mybir.dt.float32
    fp16 = mybir.dt.float16
    i16 = mybir.dt.int16
    ACT = mybir.ActivationFunctionType
    ALU = mybir.AluOpType

    tot = sum(b_sizes)
    out_cols = tot // 128

    nc = bacc.Bacc("TRN2", target_bir_lowering=False, debug=False,
                   num_devices=N_CORES, num_swdge_queues=N_QUEUES)
    a16 = nc.dram_tensor("a16", [NP, H], fp16, kind="ExternalInput")
    b16 = nc.dram_tensor("b16", [NP, H], fp16, kind="ExternalInput")
    uidx = nc.dram_tensor("uidx", [128, tot // 16], i16, kind="ExternalInput")
    midx = nc.dram_tensor("midx", [128, tot // 16], i16, kind="ExternalInput")
    wp = nc.dram_tensor("wp", [128, H + 1], f32, kind="ExternalInput")
    out = nc.dram_tensor("out", [128, out_cols], f32, kind="ExternalOutput")

    with tile.TileContext(nc) as tc:
        with (
            tc.tile_pool(name="g", bufs=3) as gpool,
            tc.tile_pool(name="t", bufs=4) as tpool,
            tc.tile_pool(name="h", bufs=4) as hpool,
            tc.tile_pool(name="sc", bufs=4) as scpool,
            tc.tile_pool(name="w", bufs=1) as wpool,
            tc.tile_pool(name="o", bufs=1) as opool,
            tc.tile_pool(name="ix", bufs=1) as idxp,
        ):
            uix = idxp.tile([128, tot // 16], i16, tag="uix")
            mix = idxp.tile([128, tot // 16], i16, tag="mix")
            nc.sync.dma_start(uix[:], uidx[:])
            nc.sync.dma_start(mix[:], midx[:])
            wsb = wpool.tile([128, H + 1], f32, tag="wsb")
            nc.sync.dma_start(wsb[:], wp[:])
            w2q4 = wpool.tile([128, 4, H], fp16, tag="w2q4")
            for q in range(4):
                nc.scalar.activation(w2q4[:, q, :], wsb[:, 0:H], ACT.Copy)
            b2c = wsb[:, H:H + 1]
            o_sb = opool.tile([128, out_cols], f32, tag="osb")
            o_sb2 = opool.tile([128, out_cols], f32, tag="osb2")

            qctr = [0]

            def body():
                off = 0
                gk = 0
                for ab in range(N_BUCKETS):
                    bsz = b_sizes[ab]
                    if bsz == 0:
                        continue
                    a, b = divmod(ab, N_CHUNKS)
                    asrc = a16[a * CHUNK:(a + 1) * CHUNK, :]
                    bsrc = b16[b * CHUNK:(b + 1) * CHUNK, :]
                    for c0 in range(0, bsz, MAX_CALL):
                        n = min(MAX_CALL, bsz - c0)
                        ug = gpool.tile([128, MAX_CALL // 128, H], fp16, tag="ug")
                        mg = gpool.tile([128, MAX_CALL // 128, H], fp16, tag="mg")
                        s0 = off + c0
                        for gt, src, ix in ((ug, asrc, uix), (mg, bsrc, mix)):
                            nc.gpsimd.dma_gather(
                                gt[:, :n // 128, :], src,
                                ix[:, s0 // 16:(s0 + n) // 16],
                                num_idxs=n, num_idxs_reg=n, elem_size=H,
                                transpose=False, single_packet=False,
                                queue_num=qctr[0] % N_QUEUES)
                            qctr[0] += 1
                        for j in range(n // BLK):
                            sl = slice(j * 4, (j + 1) * 4)
                            t = tpool.tile([128, 4, H], fp16, tag="t")
                            nc.vector.scalar_tensor_tensor(
                                t[:], ug[:, sl, :], 0.0, mg[:, sl, :],
                                op0=ALU.add, op1=ALU.add)
                            hq = hpool.tile([128, 4, H], fp16, tag="h")
                            nc.scalar.activation(hq[:], t[:], ACT.Relu)
                            p = scpool.tile([128, 4, H], fp16, tag="p")
                            nc.vector.scalar_tensor_tensor(
                                p[:], hq[:], 0.0, w2q4[:],
                                op0=ALU.add, op1=ALU.mult)
                            nc.vector.tensor_reduce(
                                o_sb[:, gk * 4:(gk + 1) * 4], p[:],
                                axis=mybir.AxisListType.X, op=ALU.add)
                            gk += 1
                    off += bsz
                nc.scalar.activation(o_sb2[:], o_sb[:], ACT.Identity,
                                     bias=b2c, scale=1.0)

            body()
            if reps > 1:
                with tc.For_i(0, reps - 1):
                    body()
            nc.sync.dma_start(out[:], o_sb2[:])
    nc.compile()
    return nc, out_cols


W_SZ = 128                    # dst window rows (PE stationary width)
N_WIN = SHARD // W_SZ         # 98 windows per core
S_GRP = 16                    # S-matrix quarters per streaming DMA


def _build_neff_b2(q_cell, reps=1):
    """Windowed-B NEFF: only the A side is gathered; B rows are expanded
    from the core's SBUF-resident slab by one-hot S-matrix matmuls.

    q_cell: tuple of 98*4 ints — 128-slot quarters per (window, src-chunk)
    cell, shared across cores.  Output: out[s % 128, s // 128] for padded
    slot s; slot column = global quarter index.
    """
    import concourse.bacc as bacc
    import concourse.mybir as mybir
    import concourse.tile as tile

    f32 = mybir.dt.float32
    fp16 = mybir.dt.float16
    i16 = mybir.dt.int16
    ACT = mybir.ActivationFunctionType
    ALU = mybir.AluOpType

    n_q = sum(q_cell)
    s_tot = n_q * 128
    out_cols = n_q
    qmax = max(q_cell)

    nc = bacc.Bacc("TRN2", target_bir_lowering=False, debug=False,
                   num_devices=N_CORES, num_swdge_queues=N_QUEUES)
    a16 = nc.dram_tensor("a16", [NP, H], fp16, kind="ExternalInput")
    bslab = nc.dram_tensor("bslab", [128, N_WIN * H], fp16, kind="ExternalInput")
    uidx = nc.dram_tensor("uidx", [128, s_tot // 16], i16, kind="ExternalInput")
    sin = nc.dram_tensor("sin", [128, s_tot], fp16, kind="ExternalInput")
    wp2 = nc.dram_tensor("wp2", [128, 2 * H + 1], f32, kind="ExternalInput")
    out = nc.dram_tensor("out", [128, out_cols], f32, kind="ExternalOutput")

    with tile.TileContext(nc) as tc:
        with (
            tc.tile_pool(name="g", bufs=4) as gpool,
            tc.tile_pool(name="s", bufs=3) as spool,
            tc.tile_pool(name="h", bufs=4) as hpool,
            tc.tile_pool(name="pm", bufs=4) as pmpool,
            tc.tile_pool(name="ps", bufs=4, space="PSUM") as pspool,
            tc.tile_pool(name="w", bufs=1) as wpool,
            tc.tile_pool(name="o", bufs=1) as opool,
            tc.tile_pool(name="ix", bufs=1) as idxp,
            tc.tile_pool(name="slab", bufs=1) as slabp,
        ):
            uix = idxp.tile([128, s_tot // 16], i16, tag="uix")
            nc.sync.dma_start(uix[:], uidx[:])
            slab = slabp.tile([128, N_WIN * H], fp16, tag="slab")
            nc.sync.dma_start(slab[:], bslab[:])
            wsb = wpool.tile([128, 2 * H + 1], f32, tag="wsb")
            nc.sync.dma_start(wsb[:], wp2[:])
            w2q = wpool.tile([128, H], fp16, tag="w2q")
            nc.scalar.activation(w2q[:], wsb[:, 0:H], ACT.Copy)
            id16 = wpool.tile([128, H], fp16, tag="id16")
            nc.scalar.activation(id16[:], wsb[:, H:2 * H], ACT.Copy)
            b2c = wsb[:, 2 * H:2 * H + 1]
            o_sb = opool.tile([128, out_cols], f32, tag="osb")
            o_sb2 = opool.tile([128, out_cols], f32, tag="osb2")

            qctr = [0]

            def body():
                stile = [None]
                qq = [0]

                def s_quarter(k):
                    # stream S tiles in groups of S_GRP quarters
                    if k % S_GRP == 0:
                        stile[0] = spool.tile([128, S_GRP * 128], fp16, tag="s",
                                              name="stile")
                        w = min(S_GRP, n_q - k) * 128
                        nc.sync.dma_start(stile[0][:, 0:w],
                                          sin[:, k * 128:k * 128 + w])
                    return stile[0][:, (k % S_GRP) * 128:(k % S_GRP + 1) * 128]

                slot = 0
                for w in range(N_WIN):
                    mini = slab[:, w * H:(w + 1) * H]
                    for a in range(N_CHUNKS):
                        q = q_cell[w * N_CHUNKS + a]
                        if q == 0:
                            continue
                        n = q * 128
                        ug = gpool.tile([128, qmax, H], fp16, tag="ug")
                        nc.gpsimd.dma_gather(
                            ug[:, :q, :], a16[a * CHUNK:(a + 1) * CHUNK, :],
                            uix[:, slot // 16:(slot + n) // 16],
                            num_idxs=n, num_idxs_reg=n, elem_size=H,
                            transpose=False, single_packet=False,
                            queue_num=qctr[0] % N_QUEUES)
                        qctr[0] += 1
                        for k in range(q):
                            s_ap = s_quarter(qq[0])
                            ps = pspool.tile([128, H], f32, tag="ps")
                            nc.tensor.matmul(ps[:], s_ap, mini,
                                             start=True, stop=False)
                            nc.tensor.matmul(ps[:], id16[:], ug[:, k, :],
                                             start=False, stop=True)
                            hq = hpool.tile([128, H], fp16, tag="h")
                            nc.scalar.activation(hq[:], ps[:], ACT.Relu)
                            pm = pmpool.tile([128, H], fp16, tag="pm")
                            nc.vector.scalar_tensor_tensor(
                                pm[:], hq[:], 0.0, w2q[:],
                                op0=ALU.add, op1=ALU.mult)
                            nc.vector.tensor_reduce(
                                o_sb[:, qq[0]:qq[0] + 1], pm[:],
                                axis=mybir.AxisListType.X, op=ALU.add)
                            qq[0] += 1
                        slot += n
                nc.scalar.activation(o_sb2[:], o_sb[:], ACT.Identity,
                                     bias=b2c, scale=1.0)

            body()
            if reps > 1:
                with tc.For_i(0, reps - 1):
                    body()
            nc.sync.dma_start(out[:], o_sb2[:])
    nc.compile()
    return nc, out_cols


def _marshal2(edge_index):
    """dst-sorted windowed marshalling.

    Core c owns dst rows [c*SHARD, (c+1)*SHARD); its edges are grouped by
    (window w = (dst % SHARD) // 128, chunk a = src // CHUNK) cells in
    (w, a) order, padded per cell to q_cell[w,a]*128 slots (q_cell shared
    across cores).  Returns q_cell and per-core uidx/sin/dstslot/inv.
    """
    src = np.asarray(edge_index[0]).astype(np.int64)
    dst = np.asarray(edge_index[1]).astype(np.int64)
    E = src.shape[0]
    core_of = dst // SHARD
    w_of = (dst % SHARD) // W_SZ
    a_of = src // CHUNK
    cell_of = w_of * N_CHUNKS + a_of
    n_cells = N_WIN * N_CHUNKS

    cnt = np.zeros((N_CORES, n_cells), dtype=np.int64)
    for c in range(N_CORES):
        m = core_of == c
        cnt[c] = np.bincount(cell_of[m], minlength=n_cells)
    q_cell = tuple(int(x) for x in -(-cnt.max(axis=0) // 128))
    n_q = int(sum(q_cell))
    s_tot = n_q * 128
    cell_base = np.concatenate([[0], np.cumsum(np.asarray(q_cell) * 128)])

    cores = []
    for c in range(N_CORES):
        m = np.nonzero(core_of == c)[0]
        order = m[np.argsort(cell_of[m], kind="stable")]
        cells_sorted = cell_of[order]
        # position of each edge within its cell
        starts = np.searchsorted(cells_sorted, np.arange(n_cells), side="left")
        within = np.arange(order.size) - starts[cells_sorted]
        slots = cell_base[cells_sorted] + within

        uloc = np.zeros(s_tot, dtype=np.int16)
        uloc[slots] = (src[order] - a_of[order] * CHUNK).astype(np.int16)
        dstrow = np.full(s_tot, -1, dtype=np.int64)
        dstrow[slots] = dst[order] % W_SZ
        inv = np.full(s_tot, -1, dtype=np.int64)
        inv[slots] = order

        sin = np.zeros((128, s_tot), dtype=np.float16)
        real = dstrow >= 0
        sin[dstrow[real], np.nonzero(real)[0]] = 1.0

        wrapped = np.ascontiguousarray(
            np.tile(uloc.reshape(s_tot // 16, 16).T, (8, 1)))
        cores.append({"uidx": wrapped, "sin": sin, "inv": inv})
    return q_cell, n_q, cores


def _prep_wp2(W2, b2):
    wp2 = np.zeros((128, 2 * H + 1), dtype=np.float32)
    wp2[:, 0:H] = np.asarray(W2, dtype=np.float32).reshape(1, H)
    wp2[:, H:2 * H] = np.eye(128, dtype=np.float32)
    wp2[:, 2 * H] = np.asarray(b2, dtype=np.float32)[0]
    return wp2


def _slab_for_core(B16, c):
    return np.ascontiguousarray(
        B16[c * SHARD:(c + 1) * SHARD]
        .reshape(N_WIN, W_SZ, H).transpose(1, 0, 2).reshape(128, N_WIN * H))


def _marshal(edge_index):
    """Bucket each core's edges; returns per-core device idx arrays and
    the info needed to invert the permutation on the host."""
    E = edge_index.shape[1]
    esh = E // N_CORES
    per_core = []
    counts = np.zeros((N_CORES, N_BUCKETS), dtype=np.int64)
    for c in range(N_CORES):
        src = np.asarray(edge_index[0, c * esh:(c + 1) * esh]).astype(np.int64)
        dst = np.asarray(edge_index[1, c * esh:(c + 1) * esh]).astype(np.int64)
        key = (src // CHUNK) * N_CHUNKS + (dst // CHUNK)
        order = np.argsort(key, kind="stable")
        sk = key[order]
        counts[c] = np.bincount(sk, minlength=N_BUCKETS)
        per_core.append((src, dst, order, sk))
    b_sizes = [int(-(-int(counts[:, ab].max()) // BLK) * BLK)
               if counts[:, ab].max() > 0 else 0 for ab in range(N_BUCKETS)]
    tot = sum(b_sizes)

    cores = []
    for c in range(N_CORES):
        src, dst, order, sk = per_core[c]
        ulocal = np.zeros(tot, dtype=np.int16)
        mlocal = np.zeros(tot, dtype=np.int16)
        inv = np.full(tot, -1, dtype=np.int64)
        off = 0
        pos = 0
        for ab in range(N_BUCKETS):
            bsz = b_sizes[ab]
            if bsz == 0:
                continue
            cnt = int(counts[c, ab])
            a, b = divmod(ab, N_CHUNKS)
            sel = order[pos:pos + cnt]
            pos += cnt
            ulocal[off:off + cnt] = (src[sel] - a * CHUNK).astype(np.int16)
            mlocal[off:off + cnt] = (dst[sel] - b * CHUNK).astype(np.int16)
            # padded slots keep idx 0 (valid row of the chunk, result unused)
            inv[off:off + cnt] = sel
            off += bsz

        def wrap(arr):
            w = arr.reshape(tot // 16, 16).T
            return np.ascontiguousarray(np.tile(w, (8, 1)))

        cores.append({"uidx": wrap(ulocal), "midx": wrap(mlocal), "inv": inv})
    return b_sizes, tot, cores


def _prep_a_inputs(user_features, movie_features, W1, b1):
    uf = np.zeros((NP, H), dtype=np.float32)
    uf[:N_NODES] = user_features
    mf = np.zeros((NP, H), dtype=np.float32)
    mf[:N_NODES] = movie_features
    wk = np.zeros((128, 3 * H), dtype=np.float16)
    wk[:, 0:H] = np.asarray(W1, dtype=np.float32)[:H].astype(np.float16)
    wk[:, H:2 * H] = np.asarray(W1, dtype=np.float32)[H:].astype(np.float16)
    wk[:, 2 * H:3 * H] = np.eye(128, dtype=np.float16)
    b1bc = np.tile(np.asarray(b1, dtype=np.float32).reshape(1, H), (128, 1))
    b1bc = np.ascontiguousarray(b1bc)
    return [{"ush": uf[c * SHARD:(c + 1) * SHARD],
             "msh": mf[c * SHARD:(c + 1) * SHARD],
             "wk": wk, "b1bc": b1bc} for c in range(N_CORES)]


def _prep_wp(W2, b2):
    wp = np.zeros((128, H + 1), dtype=np.float32)
    wp[:, 0:H] = np.asarray(W2, dtype=np.float32).reshape(1, H)
    wp[:, H] = np.asarray(b2, dtype=np.float32)[0]
    return wp


def kernel(user_features, movie_features, edge_index, W1, b1, W2, b2):
    from concourse.bass_utils import run_bass_kernel_spmd

    user_features = np.ascontiguousarray(user_features, dtype=np.float32)
    movie_features = np.ascontiguousarray(movie_features, dtype=np.float32)
    ei = np.ascontiguousarray(edge_index)
    E = ei.shape[1]

    # ---- NEFF-A: device-side A = U@W1top, B' = M@W1bot + b1 (fp16) ----
    if "A" not in _cache:
        _cache["A"] = _build_neff_a()
    nca = _cache["A"]
    in_a = _prep_a_inputs(user_features, movie_features, W1, b1)
    res_a = run_bass_kernel_spmd(nca, in_a, core_ids=list(range(N_CORES)))
    A16 = np.concatenate([res_a.results[c]["a16o"] for c in range(N_CORES)])
    B16 = np.concatenate([res_a.results[c]["b16o"] for c in range(N_CORES)])

    # ---- host marshalling of edges (dst-sorted windowed cells) ----
    q_cell, n_q, cores = _marshal2(ei)

    key_b = ("B2", q_cell)
    if key_b not in _cache:
        _cache[key_b] = _build_neff_b2(q_cell)
    ncb, out_cols = _cache[key_b]

    wp2 = _prep_wp2(W2, b2)
    in_b = [{"a16": A16, "bslab": _slab_for_core(B16, c),
             "uidx": cores[c]["uidx"], "sin": cores[c]["sin"], "wp2": wp2}
            for c in range(N_CORES)]
    res_b = run_bass_kernel_spmd(ncb, in_b, core_ids=list(range(N_CORES)))

    # ---- host inverse permutation ----
    # padded-stream slot s lives at device out[s % 128, s // 128]
    out = np.empty(E, dtype=np.float32)
    s = np.arange(n_q * 128)
    flat_pos = (s % 128) * out_cols + s // 128
    for c in range(N_CORES):
        vals = res_b.results[c]["out"].reshape(-1)[flat_pos]
        inv = cores[c]["inv"]
        mask = inv >= 0
        out[inv[mask]] = vals[mask]
    return out

